# revision 6
# baseline (speedup 1.0000x reference)
"""Canny filter (blur -> sobel -> orientation-quantized NMS) on 8 Trainium2 cores.

Self-contained: batch 16 is sharded 2 images/core (pure data parallel);
each core runs an identical Bass/Tile program on its [2,3,512,512] slice.

Both images are processed side by side in the free dimension of every tile
([128 rows x 2*(512+2) cols]). Per 128-row tile:
  - channel-summed load (1 HWDGE + 2 SWDGE accumulate DMAs) into a
    column-padded tile
  - full 3x3 gauss as 3 matmuls/image on PE (vertical taps in the banded
    128x128 lhsT, horizontal taps as free-dim-shifted rhs), sobel as 5
    banded matmuls/image
  - NMS with 5 fused custom-DVE ops (registered into dve_ops.OPS below):
      CANNY_MSQ  mc  = gx^2 + gy^2
      CANNY_Q    q   = (gx*gy > 0) != (T3*gx^2 < gy^2)   (diag handedness)
      CANNY_M1N  m1n = min(gx^2,gy^2)*K3 >= mc           (N/S class)
      CANNY_M2   m2  = min(gx^2,gy^2)*K4 <  mc           (E/W class)
      CANNY_NMS  tn2 = (ad >= mc) ? 0 : mc               (verdict + zero)
    using the min-folded class identities  N/S <=> p*(1+tan^2 56.25) >= msq,
    E/W <=> p*(1+tan^2 78.75) < msq,  p = min(gx^2, gy^2).
    The class merge is 3 copy_predicated ops into the NW/SE pair-max tile.
NMS neighbor rows come from SBUF->SBUF DMA row-shifted copies of msq.
"""
import sys
import numpy as np

sys.path.insert(0, "/opt/trn_rl_repo")

import concourse.bacc as bacc
import concourse.tile as tile
from concourse import mybir
from concourse.bass_utils import run_bass_kernel_spmd
from contextlib import ExitStack

from concourse.dve_spec import (
    Spec, Src0, Src1, C0, Zero, lower as _dve_lower, minn, select, ne, sq,
)
import concourse.dve_ops as _dvo
from concourse.dve_uop import DveOpSpec as _DveOpSpec

F32 = mybir.dt.float32
U8 = mybir.dt.uint8

B, C, H, W = 16, 3, 512, 512
N_CORES = 8
B_PER = B // N_CORES          # 2 images per core
P = 128                       # partitions per tile
WP = W + 2                    # padded width per image
FW = B_PER * W                # 1024: free width of unpadded working tiles
FWP = B_PER * WP              # 1028: free width of padded tiles
# row-tile input origins per image; tile t covers input rows [R, R+128),
# valid output rows are [R+3, min(R+125, 512))
R_INS = [-3, 119, 241, 363, 485]

_ALU = mybir.AluOpType
_ACTF = mybir.ActivationFunctionType

# min-folded class thresholds (see module docstring)
_K3 = float(1.0 + np.tan(np.deg2rad(56.25)) ** 2)   # N/S:  p*K3 >= msq
_K4 = float(1.0 + np.tan(np.deg2rad(78.75)) ** 2)   # E/W:  p*K4 <  msq
_T3 = float(np.tan(np.deg2rad(56.25)) ** 2)         # diag handedness helper


def _reg_dve(name, spec):
    """Register a custom DVE op with programmatically pinned uops shas."""
    if name in _dvo._SUB_OPCODE_FOR_NAME:
        return next(o for o in _dvo.OPS if o.name == name)
    shas = {}
    for ver in ("v3", "v4"):
        uops = _dve_lower(spec, ver=ver)
        shas[ver] = _DveOpSpec(name=name, opcode=1, uops=uops).sha(ver)
    op = _dvo.DveOp(name, spec, subdim=False, uops_sha=shas)
    _dvo.OPS.append(op)
    _dvo._SUB_OPCODE_FOR_NAME[name] = _dvo._CUSTOM_DVE_ROW_BASE + len(_dvo.OPS) - 1
    return op


_OP_MSQ = _reg_dve("CANNY_MSQ", Spec(
    body=sq(Src0) + sq(Src1),
    reference=lambda in0, in1, s0, s1, imm2:
        (in0 * in0 + in1 * in1).astype(np.float32)))
_OP_Q = _reg_dve("CANNY_Q", Spec(
    body=ne((Src0 * Src1) > Zero, (sq(Src0) * C0) < sq(Src1)),
    reference=lambda in0, in1, s0, s1, imm2:
        (((in0 * in1) > 0) != ((in0 * in0 * s0) < (in1 * in1))).astype(np.float32)))
_OP_M1N = _reg_dve("CANNY_M1N", Spec(
    body=(minn(sq(Src0), sq(Src1)) * C0) >= (sq(Src0) + sq(Src1)),
    reference=lambda in0, in1, s0, s1, imm2:
        ((np.minimum(in0 * in0, in1 * in1) * s0) >= (in0 * in0 + in1 * in1)
         ).astype(np.float32)))
_OP_M2 = _reg_dve("CANNY_M2", Spec(
    body=(minn(sq(Src0), sq(Src1)) * C0) < (sq(Src0) + sq(Src1)),
    reference=lambda in0, in1, s0, s1, imm2:
        ((np.minimum(in0 * in0, in1 * in1) * s0) < (in0 * in0 + in1 * in1)
         ).astype(np.float32)))
_OP_NMS = _reg_dve("CANNY_NMS", Spec(
    body=select(Src0 >= Src1, Zero, Src1),
    reference=lambda in0, in1, s0, s1, imm2:
        np.where(in0 >= in1, 0.0, in1).astype(np.float32)))


def _banded(diag_vals, scale=1.0, fold_top_row=None, fold_bot_row=None):
    """lhsT[k, m] for out[m] = sum_dk w[dk] * in[m+dk], dk in {-1,0,1}.

    fold_top_row=r: in[r-1] := in[r] for out row r (blurred-field replication)
    fold_bot_row=r: in[r+1] := in[r] for out row r
    """
    wm1, w0, wp1 = [v * scale for v in diag_vals]
    A = np.zeros((P, P), np.float64)
    for m in range(P):
        for dk, wv in ((-1, wm1), (0, w0), (1, wp1)):
            k = m + dk
            if 0 <= k < P and wv != 0.0:
                A[k, m] += wv
    if fold_top_row is not None:
        r = fold_top_row
        A[r - 1, r] = 0.0
        A[r, r] += wm1
    if fold_bot_row is not None:
        r = fold_bot_row
        A[r + 1, r] = 0.0
        A[r, r] += wp1
    return A.astype(np.float32)


def _build_weights():
    v = np.array([np.exp(-0.5), 1.0, np.exp(-0.5)], np.float64)
    sv = v.sum()
    g1 = v / sv                      # vertical gaussian taps
    h1 = 1.0 / (3.0 * sv)            # center horizontal tap (folds the /C)
    r0 = float(np.exp(-0.5))         # outer/center horizontal tap ratio
    ws = {}
    # fused 3x3 gauss: tt = sum_dc r_dc * Vg @ cv[c+dc], r = [r0, 1, r0];
    # tt is blur/h1 and the sobel matrices below are pre-scaled by h1.
    ws["Vg"] = _banded((g1[0], g1[1], g1[2]))
    ws["VgR"] = r0 * ws["Vg"]
    # sobel x2 (integer taps): Gx = [1,2,1]_v (x) [-1,0,1]_h,
    # Gy = [-1,0,1]_v (x) [1,2,1]_h
    for suf, kw in (("", {}), ("_t", {"fold_top_row": 3}),
                    ("_b", {"fold_bot_row": 26})):
        ws["Vs" + suf] = _banded((1.0, 2.0, 1.0), h1, **kw)
        ws["Vsn" + suf] = -ws["Vs" + suf]
        ws["Vd" + suf] = _banded((-1.0, 0.0, 1.0), h1, **kw)
        ws["Vd2" + suf] = 2.0 * ws["Vd" + suf]
    return ws


_NC_CACHE = None


def _build_program():
    nc = bacc.Bacc("TRN2", target_bir_lowering=False, debug=False, num_devices=N_CORES)
    img = nc.declare_dram_parameter("img", [B_PER, C, H, W], F32, isOutput=False)
    out = nc.declare_dram_parameter("out", [B_PER, 1, H, W], F32, isOutput=True)

    wnp = _build_weights()
    wkeys = sorted(wnp.keys())
    wcat = np.concatenate([wnp[k] for k in wkeys], axis=1)   # [128, 14*128]
    wdram_all = nc.inline_tensor(wcat, name="w_all")

    with tile.TileContext(nc) as tc, ExitStack() as ctx:
        cpool = ctx.enter_context(tc.tile_pool(name="consts", bufs=1))
        ctpool = ctx.enter_context(tc.tile_pool(name="cts", bufs=3))
        pool = ctx.enter_context(tc.tile_pool(name="work", bufs=2))
        mpool = ctx.enter_context(tc.tile_pool(name="masks", bufs=2))
        pp_t = ctx.enter_context(tc.tile_pool(name="pst", bufs=2, space="PSUM"))
        pp_g = ctx.enter_context(tc.tile_pool(name="psg", bufs=1, space="PSUM"))

        wall = cpool.tile([P, len(wkeys) * P], F32, tag="w_all")
        nc.sync.dma_start(wall[:], wdram_all[:])
        wsb = {k: wall[:, j * P:(j + 1) * P] for j, k in enumerate(wkeys)}
        zero = cpool.tile([P, FWP], F32, tag="zero")
        nc.vector.memset(zero[:], 0.0)

        # persistent ping-pong buffers: pads/edges zeroed once
        msqs, nsbs, ssbs = [], [], []
        for j in range(2):
            mq = cpool.tile([P, FWP], F32, tag=f"msq{j}")
            mv = mq[:, :].rearrange("p (i w) -> p i w", i=B_PER)
            nc.vector.memset(mv[:, :, 0:WP:WP - 1], 0.0)   # cols 0,513 per image
            msqs.append(mq)
            nsb = cpool.tile([P, FWP], F32, tag=f"nsb{j}")
            nc.vector.memset(nsb[:], 0.0)   # edge rows are never DMA-written
            nsbs.append(nsb)
            ssb = cpool.tile([P, FWP], F32, tag=f"ssb{j}")
            nc.vector.memset(ssb[:], 0.0)
            ssbs.append(ssb)

        def im3(t):
            return t[:, :].rearrange("p (i w) -> p i w", i=B_PER)

        def shifted(base, off):
            # 2D-free AP: both images' [off, off+512) windows of a padded tile
            return im3(base)[:, :, off:off + W]

        def stage_load(t_i, R):
            # channel-summed load into a column-padded tile: base DMA carries
            # channel 0, then two SWDGE accumulate-DMAs add channels 1 and 2
            # in the DMA datapath. Image cols land at [1, 513).
            top = t_i == 0
            bot = t_i == len(R_INS) - 1
            ct = ctpool.tile([P, FWP], F32, tag="ct")
            cv = ct[:, :].rearrange("p (i w) -> p i w", i=B_PER)
            cc = cv[:, :, 1:1 + W]
            if top:
                for i in range(B_PER):
                    nc.sync.dma_start(cc[0:3, i, :],
                                      img[i, 0, 0:1, :].broadcast_to((3, W)))
                    for k in (1, 2):
                        nc.gpsimd.dma_start(cc[0:3, i, :],
                                            img[i, k, 0:1, :].broadcast_to((3, W)),
                                            accum_op=_ALU.add)
                nc.sync.dma_start(cc[3:128, :, :],
                                  img[:, 0, 0:125, :].rearrange("i p w -> p i w"))
                for k in (1, 2):
                    nc.gpsimd.dma_start(cc[3:128, :, :],
                                        img[:, k, 0:125, :].rearrange("i p w -> p i w"),
                                        accum_op=_ALU.add)
            elif bot:
                # R=485: rows 0..26 <- img 485..511, rows 27..29 <- img 511
                nc.sync.dma_start(cc[0:27, :, :],
                                  img[:, 0, R:R + 27, :].rearrange("i p w -> p i w"))
                for k in (1, 2):
                    nc.gpsimd.dma_start(cc[0:27, :, :],
                                        img[:, k, R:R + 27, :].rearrange("i p w -> p i w"),
                                        accum_op=_ALU.add)
                for i in range(B_PER):
                    nc.sync.dma_start(cc[27:30, i, :],
                                      img[i, 0, 511:512, :].broadcast_to((3, W)))
                    for k in (1, 2):
                        nc.gpsimd.dma_start(cc[27:30, i, :],
                                            img[i, k, 511:512, :].broadcast_to((3, W)),
                                            accum_op=_ALU.add)
            else:
                nc.sync.dma_start(cc[:, :, :],
                                  img[:, 0, R:R + 128, :].rearrange("i p w -> p i w"))
                for k in (1, 2):
                    nc.gpsimd.dma_start(cc[:, :, :],
                                        img[:, k, R:R + 128, :].rearrange("i p w -> p i w"),
                                        accum_op=_ALU.add)
            # replicate edge columns into the pads (after the channel sum)
            nc.scalar.copy(cv[:, :, 0:WP:WP - 1], cv[:, :, 1:WP:W - 1])
            return ct

        def stage_a(t_i, R, ct):
            top = t_i == 0
            bot = t_i == len(R_INS) - 1
            suf = "_t" if top else ("_b" if bot else "")
            nr = 27 if bot else 125          # last valid output row + 1

            # ---- fused 3x3 gauss (PE): 3 shifted matmuls per image
            ps_t = pp_t.tile([P, FW], F32, tag="t")
            for i in range(B_PER):
                o = i * WP
                s_ = slice(i * W, (i + 1) * W)
                nc.tensor.matmul(ps_t[:, s_], wsb["VgR"], ct[:, o:o + W],
                                 start=True, stop=False)
                nc.tensor.matmul(ps_t[:, s_], wsb["Vg"], ct[:, o + 1:o + 1 + W],
                                 start=False, stop=False)
                nc.tensor.matmul(ps_t[:, s_], wsb["VgR"], ct[:, o + 2:o + 2 + W],
                                 start=False, stop=True)
            tt = pool.tile([P, FWP], F32, tag="tt")
            nc.scalar.copy(shifted(tt, 1), im3(ps_t))
            nc.scalar.copy(im3(tt)[:, :, 0:WP:WP - 1], im3(tt)[:, :, 1:WP:W - 1])

            # ---- sobel (PE)
            ps_gx = pp_g.tile([P, FW], F32, tag="gx")
            ps_gy = pp_g.tile([P, FW], F32, tag="gy")
            for i in range(B_PER):
                o = i * WP
                s_ = slice(i * W, (i + 1) * W)
                nc.tensor.matmul(ps_gx[:, s_], wsb["Vsn" + suf], tt[:, o:o + W],
                                 start=True, stop=False)
                nc.tensor.matmul(ps_gx[:, s_], wsb["Vs" + suf], tt[:, o + 2:o + 2 + W],
                                 start=False, stop=True)
                nc.tensor.matmul(ps_gy[:, s_], wsb["Vd" + suf], tt[:, o:o + W],
                                 start=True, stop=False)
                nc.tensor.matmul(ps_gy[:, s_], wsb["Vd2" + suf], tt[:, o + 1:o + 1 + W],
                                 start=False, stop=False)
                nc.tensor.matmul(ps_gy[:, s_], wsb["Vd" + suf], tt[:, o + 2:o + 2 + W],
                                 start=False, stop=True)

            # ---- one PSUM stream allowed per DVE op: stage gx in SBUF (ACT)
            gxs = pool.tile([P, FW], F32, tag="gxs")
            nc.scalar.copy(gxs[:], ps_gx[:])

            # ---- fused custom-DVE ops from (gx, gy)
            msq = msqs[t_i % 2]
            mc = shifted(msq, 1)
            nc.vector._custom_dve(_OP_MSQ, out=mc, in0=im3(gxs), in1=im3(ps_gy))

            # ---- N/S row-shifted copies of msq (SBUF->SBUF DMA, pads incl.)
            nsb, ssb = nsbs[t_i % 2], ssbs[t_i % 2]
            nc.sync.dma_start(nsb[3:nr, :], msq[2:nr - 1, :])
            nc.sync.dma_start(ssb[2:nr, :], msq[3:nr + 1, :])
            if top:
                nc.sync.dma_start(nsb[3:4, :], zero[0:1, :])
            if bot:
                nc.sync.dma_start(ssb[26:27, :], zero[0:1, :])

            q = mpool.tile([P, FW], U8, tag="q")
            nc.vector._custom_dve(_OP_Q, out=q[:], in0=gxs[:], in1=ps_gy[:], s0=_T3)
            m1n = mpool.tile([P, FW], U8, tag="m1n")
            nc.vector._custom_dve(_OP_M1N, out=m1n[:], in0=gxs[:], in1=ps_gy[:], s0=_K3)
            m2 = mpool.tile([P, FW], U8, tag="m2")
            nc.vector._custom_dve(_OP_M2, out=m2[:], in0=gxs[:], in1=ps_gy[:], s0=_K4)
            return dict(q=q, m1n=m1n, m2=m2, msq=msq, mc=mc, nsb=nsb, ssb=ssb,
                        R=R, nr=nr)

        def stage_b(st):
            q, m1n, m2 = st["q"], st["m1n"], st["m2"]
            msq, mc, nsb, ssb = st["msq"], st["mc"], st["nsb"], st["ssb"]

            a1 = pool.tile([P, FW], F32, tag="a1")   # NE / SW
            nc.vector.tensor_tensor(im3(a1), shifted(nsb, 2), shifted(ssb, 0), _ALU.max)
            ad = pool.tile([P, FW], F32, tag="ad")   # NW / SE -> merge target
            nc.vector.tensor_tensor(im3(ad), shifted(nsb, 0), shifted(ssb, 2), _ALU.max)
            a2 = pool.tile([P, FW], F32, tag="a2")   # N / S
            nc.vector.tensor_tensor(im3(a2), shifted(nsb, 1), shifted(ssb, 1), _ALU.max)
            a0 = pool.tile([P, FW], F32, tag="a0")   # E / W
            nc.vector.tensor_tensor(im3(a0), shifted(msq, 0), shifted(msq, 2), _ALU.max)

            nc.vector.copy_predicated(ad[:], q[:], a1[:])
            nc.vector.copy_predicated(ad[:], m1n[:], a2[:])
            nc.vector.copy_predicated(ad[:], m2[:], a0[:])

            tn2 = pool.tile([P, FW], F32, tag="tn2")
            nc.vector._custom_dve(_OP_NMS, out=im3(tn2), in0=im3(ad), in1=mc)
            osb = pool.tile([P, FW], F32, tag="osb")
            nc.scalar.activation(osb[:], tn2[:], _ACTF.Sqrt, scale=0.25)

            r0, r1 = st["R"] + 3, st["R"] + st["nr"]
            nc.sync.dma_start(
                out[:, 0, r0:r1, :].rearrange("i r w -> r i w"),
                osb[3:st["nr"], :].rearrange("p (i w) -> p i w", i=B_PER))

        # software-pipelined emission:
        #   load(k+2) and stage A(k+1) are emitted before stage B(k)
        n = len(R_INS)
        cts = [None] * n
        cts[0] = stage_load(0, R_INS[0])
        cts[1] = stage_load(1, R_INS[1])
        pending = None
        for t_i, R in enumerate(R_INS):
            if t_i + 2 < n:
                cts[t_i + 2] = stage_load(t_i + 2, R_INS[t_i + 2])
            st = stage_a(t_i, R, cts[t_i])
            if pending is not None:
                stage_b(pending)
            pending = st
        stage_b(pending)

    nc.compile()
    return nc


def _get_program():
    global _NC_CACHE
    if _NC_CACHE is None:
        _NC_CACHE = _build_program()
    return _NC_CACHE


def kernel(img, w_gauss=None, w_sobel_x=None, w_sobel_y=None, w_dir=None):
    img = np.ascontiguousarray(np.asarray(img, dtype=np.float32))
    assert img.shape == (B, C, H, W)
    nc = _get_program()
    in_maps = [{"img": img[c * B_PER:(c + 1) * B_PER]} for c in range(N_CORES)]
    res = run_bass_kernel_spmd(nc, in_maps, list(range(N_CORES)))
    return np.concatenate([res.results[c]["out"] for c in range(N_CORES)], axis=0)
